# revision 11
# baseline (speedup 1.0000x reference)
"""nn_Attention_19121194402320 on 8 TRN2 NeuronCores (raw Bass, bf16).

The reference module is

    k = (key @ Wk.T).reshape(B, H, S, D)       # RAW reshape
    q, v analogously
    attn = softmax(q @ k.T, axis=-1)
    out  = einsum('bnqk,bnvd->bnqd', attn, v)  # NOTE the 'k' vs 'v' labels
    out.transpose(0,2,1,3).reshape(B, S, E)

The second einsum's contraction labels differ ('k' in the first operand,
'v' in the second), so einsum sums each independently:

    out[b,n,q,d] = (sum_k attn[b,n,q,k]) * (sum_v v[b,n,v,d])
                 = sum_v v[b,n,v,d]          (softmax rows sum to 1)

i.e. every output row (for any q) equals the per-head column-sum of the
raw-reshaped V projection; query/key/Wq/Wk do not affect the output.

Math: with Y = value[b] @ Wv.T ([1024, 768]), raw-reshape head n covers
flat chunks g in [1024n, 1024(n+1)); chunk g = 12s + c is Y[s, 64c:64c+64].
So r_b[64n+d] = sum_c sum_{s in S(n,c)} Y[s, 64c+d] where S(n,c) =
[ceil((1024n-c)/12), ceil((1024(n+1)-c)/12)).  The boundary of S(n,.) as a
function of c moves by AT MOST ONE ROW: lo(n,c) = m_n + [c < theta_n] with
m_n = floor(1024n/12), theta_n = 4 if n%3==1, 8 if n%3==2, else no shift.
Hence with base segments [m_n, m_{n+1}) (indicator U [1024, 12]):

    Zb[n,:]  = sum_{s in base seg n} X[s,:]
    rbase    = Zb @ Wsum,   Wsum[e,d]   = sum_{c<12} Wv.T[e, 64c+d]
    y_n      = X[m_n] @ Wpre_{theta_n}, Wpre_t[e,d] = sum_{c<t} Wv.T[e, 64c+d]
    r[n]     = rbase[n] - y_n*[n has bnd] + y_{n+1}*[n+1 has bnd]

(verified to 3e-7 vs the fp32 jax reference).

Sharding: by the contraction dim e — core k owns e-slice [96k, 96k+96).
Each core loads: its column slice of value for ALL 4 batches (host-packed
[128, 32*96] bf16, 786 KB), plus an 80 KB aux tensor (U mask tiles, the
three 96x64 W matrices, and the 8 transposed boundary rows per batch).
It returns a [48, 192] fp32 partial (rbase | y4 | y8); the host sums the
8 partials, applies the +-y corrections, and tiles rows to (B, S, E).
This nearly halves per-core HBM traffic vs loading full Wv per core
(the previous layout) and eliminates the long gather/broadcast tail.

Device pipeline per core (scalar issues all DMAs — its post-prologue
path is ~0.6 us faster than sync's, and sequential DMAs on one HWDGE
queue drain FIFO, so the first half's semaphore fires early):
  scalar : aux DMA, xc half DMAs (2), out DMA
  PE     : y4,y8 correction mms; 32 x (LDW + 12-col MM) base segment
           sums accumulated per batch in its own PSUM bank; one fused
           [96,48]@[96,64] rbase mm.  lhsT reads 128 cols (96 data + 32
           spill into the next tile) so the compiler's Fast-Weight-Load
           kicks in; spill lands in psum partitions 96:127, never read.
  DVE    : psum->sbuf bf16 casts of Zb.T, fp32 copies of the outputs.
"""

from contextlib import ExitStack

import ml_dtypes
import numpy as np

import concourse.bass as bass
from concourse import bacc, mybir
from concourse.bass_utils import run_bass_kernel_spmd

B, S, E, H, D = 4, 1024, 768, 12, 64
EW = 96              # e-slice width per core
NT = 32              # s-tiles of 128 rows (4 batches x 8)
XC = NT * EW         # 3072 xc columns
FP = mybir.dt.float32
BF = mybir.dt.bfloat16

LOB = [(1024 * n) // 12 for n in range(13)]          # base segment bounds
M4 = [LOB[n] for n in (1, 4, 7, 10)]                 # theta=4 boundary rows
M8 = [LOB[n] for n in (2, 5, 8, 11)]                 # theta=8 boundary rows

_CACHE = {}


def _build_nc():
    # Bass.__init__ unconditionally emits 4 const-tile memsets (gpsimd) and a
    # full all-engine barrier before user code; this kernel uses neither
    # (no const-bias activations, all cross-engine deps via explicit sems),
    # so suppress them during construction to shave NEFF startup time.
    _memset = bass.BassGpSimd.memset
    _barrier = bass.Bass.all_engine_barrier
    bass.BassGpSimd.memset = lambda self, ap, c: None
    bass.Bass.all_engine_barrier = lambda self, **kw: None
    try:
        nc = bacc.Bacc("TRN2", target_bir_lowering=False, debug=False,
                       enable_partition_id=False)
    finally:
        bass.BassGpSimd.memset = _memset
        bass.Bass.all_engine_barrier = _barrier

    xc_d = nc.dram_tensor("xc", [128, XC], BF, kind="ExternalInput").ap()
    aux_d = nc.dram_tensor("aux", [128, 320], BF, kind="ExternalInput").ap()
    out_d = nc.dram_tensor("out", [48, 192], FP, kind="ExternalOutput").ap()

    xc_sb = nc.alloc_sbuf_tensor("xc_sb", [128, XC], BF).ap()
    aux_sb = nc.alloc_sbuf_tensor("aux_sb", [128, 320], BF).ap()
    zbt_sb = nc.alloc_sbuf_tensor("zbt_sb", [96, 48], BF).ap()
    out_sb = nc.alloc_sbuf_tensor("out_sb", [48, 192], FP).ap()

    # aux column map
    UM = slice(0, 96)            # U mask tiles: col st*12+n
    WSUM = slice(96, 160)
    WP4 = slice(160, 224)
    WP8 = slice(224, 288)
    XR4 = slice(288, 304)        # col b*4+i, boundary row M4[i] of batch b
    XR8 = slice(304, 320)

    with ExitStack() as ctx:
        # one bank per batch: psum accumulation groups are tracked per 2KB
        # bank region, so concurrent per-batch chains must not share a bank
        pz = [ctx.enter_context(nc.psum_tensor(f"pz{b}", [128, 512], FP))
              for b in range(4)]
        pr = ctx.enter_context(nc.psum_tensor("pr", [128, 512], FP))
        daux = ctx.enter_context(nc.semaphore("daux"))
        dx1 = ctx.enter_context(nc.semaphore("dx1"))
        dx2 = ctx.enter_context(nc.semaphore("dx2"))
        dout = ctx.enter_context(nc.semaphore("dout"))
        pe_sem = ctx.enter_context(nc.semaphore("pe_sem"))
        dve_sem = ctx.enter_context(nc.semaphore("dve_sem"))
        dcopy = ctx.enter_context(nc.semaphore("dcopy"))
        msem = ctx.enter_context(nc.semaphore("msem"))
        block = ctx.enter_context(nc.Block(no_gpsimd_drain=True))

        @block.scalar
        def _(scalar: bass.BassEngine):
            scalar.dma_start(aux_sb, aux_d).then_inc(daux, 16)
            scalar.dma_start(xc_sb[:, 0:XC // 2], xc_d[:, 0:XC // 2]
                             ).then_inc(dx1, 16)
            scalar.dma_start(xc_sb[:, XC // 2:XC], xc_d[:, XC // 2:XC]
                             ).then_inc(dx2, 16)
            scalar.wait_ge(dcopy, 1)
            scalar.dma_start(out_d, out_sb).then_inc(dout, 16)
            scalar.wait_ge(dout, 16)

        @block.tensor
        def _(tensor: bass.BassEngine):
            tensor.wait_ge(daux, 16)
            # corrections first: depend only on aux
            nc.tensor.matmul(pr[0:16, 256:320], aux_sb[0:96, XR4],
                             aux_sb[0:96, WP4], start=True, stop=True)
            nc.tensor.matmul(pr[0:16, 320:384], aux_sb[0:96, XR8],
                             aux_sb[0:96, WP8], start=True, stop=True
                             ).then_inc(pe_sem)                    # pe=1
            # base segment sums Zb.T, accumulated per batch
            for b in range(4):
                if b == 0:
                    tensor.wait_ge(dx1, 16)
                elif b == 2:
                    tensor.wait_ge(dx2, 16)
                for st in range(8):
                    t = b * 8 + st
                    # 128-wide lhsT (32-col spill into the next tile) turns
                    # on FWL; the spill only pollutes psum partitions
                    # 96:127, which are never read.  The last tile of each
                    # DMA half must not spill across the half boundary.
                    w = 96 if t in (15, 31) else 128
                    mm = nc.tensor.matmul(
                        pz[b][0:w, 0:12],
                        xc_sb[:, t * EW:t * EW + w],
                        aux_sb[:, st * 12:(st + 1) * 12],
                        start=(st == 0), stop=(st == 7))
                    if st == 7:
                        mm.then_inc(pe_sem)                        # pe=2+b
            # rbase for all 4 batches in one mm: out rows (b*12+n)
            tensor.wait_ge(dve_sem, 4)
            nc.tensor.matmul(pr[0:48, 0:64], zbt_sb[:, 0:48],
                             aux_sb[0:96, WSUM], start=True, stop=True
                             ).then_inc(pe_sem)                    # pe=6

        @block.vector
        def _(vector: bass.BassEngine):
            nc.vector.memset(out_sb, 0.0).then_inc(msem)
            vector.wait_ge(msem, 1)
            vector.wait_ge(pe_sem, 1)
            nc.vector.tensor_copy(out_sb[0:16, 64:192], pr[0:16, 256:384])
            for b in range(4):
                vector.wait_ge(pe_sem, 2 + b)
                nc.vector.tensor_copy(zbt_sb[:, b * 12:(b + 1) * 12],
                                      pz[b][0:96, 0:12]
                                      ).then_inc(dve_sem)
            vector.wait_ge(pe_sem, 6)
            nc.vector.tensor_copy(out_sb[0:48, 0:64], pr[0:48, 0:64]
                                  ).then_inc(dcopy)

    nc.compile()
    return nc


def _get_nc():
    if "nc" not in _CACHE:
        _CACHE["nc"] = _build_nc()
    return _CACHE["nc"]


def _umask_tiles() -> np.ndarray:
    """um[p, st*12+n] = 1 iff base segment n contains row st*128+p."""
    um = np.zeros((128, 96), np.float32)
    for st in range(8):
        for n in range(12):
            for p in range(128):
                s = st * 128 + p
                if LOB[n] <= s < LOB[n + 1]:
                    um[p, st * 12 + n] = 1.0
    return um


def _in_maps(inputs):
    v = np.asarray(inputs["value"], dtype=np.float32)
    WT = np.asarray(inputs["Wv"], np.float32).T          # [E, E]
    Wg = WT.reshape(E, 12, 64)
    wsum = Wg.sum(1)
    wp4 = Wg[:, :4, :].sum(1)
    wp8 = Wg[:, :8, :].sum(1)
    um = _umask_tiles()

    maps = []
    for k in range(8):
        sl = slice(k * EW, (k + 1) * EW)
        # xc[p, (b*8+st)*96 + e] = value[b, st*128+p, 96k+e]
        xc = (v[:, :, sl].reshape(4, 8, 128, EW)
              .transpose(2, 0, 1, 3).reshape(128, XC))
        aux = np.zeros((128, 320), np.float32)
        aux[:, 0:96] = um
        aux[0:96, 96:160] = wsum[sl]
        aux[0:96, 160:224] = wp4[sl]
        aux[0:96, 224:288] = wp8[sl]
        # xr: col b*4+i = boundary row M[i] of batch b, e on partitions
        aux[0:96, 288:304] = v[:, M4, sl].reshape(16, EW).T
        aux[0:96, 304:320] = v[:, M8, sl].reshape(16, EW).T
        maps.append({
            "xc": np.ascontiguousarray(xc).astype(ml_dtypes.bfloat16),
            "aux": aux.astype(ml_dtypes.bfloat16),
        })
    return maps


def _assemble(results):
    # sum the 8 e-slice partials, then apply boundary corrections
    acc = np.zeros((48, 192), np.float64)
    for k in range(8):
        acc += results[k]["out"].astype(np.float64)
    rbase = acc[0:48, 0:64].reshape(4, 12, 64)           # [b, n, d]
    y4 = acc[0:16, 64:128].reshape(4, 4, 64)             # [b, i, d]
    y8 = acc[0:16, 128:192].reshape(4, 4, 64)

    r = rbase.copy()
    for i, n in enumerate((1, 4, 7, 10)):
        r[:, n] -= y4[:, i]
        r[:, n - 1] += y4[:, i]
    for i, n in enumerate((2, 5, 8, 11)):
        r[:, n] -= y8[:, i]
        r[:, n - 1] += y8[:, i]

    row = r.reshape(B, E).astype(np.float32)             # [b, 768]
    out = np.empty((B, S, E), np.float32)
    out[:] = row[:, None, :]
    return out


def run(inputs, trace=False, **kw):
    """Run on hardware; returns (full_output, BassKernelResults)."""
    nc = _get_nc()
    res = run_bass_kernel_spmd(nc, _in_maps(inputs), core_ids=list(range(8)),
                               trace=trace, **kw)
    return _assemble(res.results), res


def kernel(**inputs) -> np.ndarray:
    out, _ = run(inputs)
    return out


# revision 14
# speedup vs baseline: 1.1820x; 1.1820x over previous
"""nn_Attention_19121194402320 on 8 TRN2 NeuronCores (raw Bass, bf16).

The reference module is

    k = (key @ Wk.T).reshape(B, H, S, D)       # RAW reshape
    q, v analogously
    attn = softmax(q @ k.T, axis=-1)
    out  = einsum('bnqk,bnvd->bnqd', attn, v)  # NOTE the 'k' vs 'v' labels
    out.transpose(0,2,1,3).reshape(B, S, E)

The second einsum's contraction labels differ ('k' in the first operand,
'v' in the second), so einsum sums each independently:

    out[b,n,q,d] = (sum_k attn[b,n,q,k]) * (sum_v v[b,n,v,d])
                 = sum_v v[b,n,v,d]          (softmax rows sum to 1)

i.e. every output row (for any q) equals the per-head column-sum of the
raw-reshaped V projection; query/key/Wq/Wk do not affect the output.

Math: with Y = value[b] @ Wv.T ([1024, 768]), raw-reshape head n covers
flat chunks g in [1024n, 1024(n+1)); chunk g = 12s + c is Y[s, 64c:64c+64].
So r_b[64n+d] = sum_c sum_{s in S(n,c)} Y[s, 64c+d] where S(n,c) =
[ceil((1024n-c)/12), ceil((1024(n+1)-c)/12)).  The boundary of S(n,.) as a
function of c moves by AT MOST ONE ROW: lo(n,c) = m_n + [c < theta_n] with
m_n = floor(1024n/12), theta_n = 4 if n%3==1, 8 if n%3==2, else no shift.
Hence with base segments [m_n, m_{n+1}) (indicator U [1024, 12]):

    Zb[n,:]  = sum_{s in base seg n} X[s,:]
    rbase    = Zb @ Wsum,   Wsum[e,d]   = sum_{c<12} Wv.T[e, 64c+d]
    y_n      = X[m_n] @ Wpre_{theta_n}, Wpre_t[e,d] = sum_{c<t} Wv.T[e, 64c+d]
    r[n]     = rbase[n] - y_n*[n has bnd] + y_{n+1}*[n+1 has bnd]

(verified to 3e-7 vs the fp32 jax reference).

Sharding: by the contraction dim e — core k owns e-slice [96k, 96k+96).
Each core loads: its column slice of value for ALL 4 batches (host-packed
[128, 32*96] bf16, 786 KB), plus an 80 KB aux tensor (U mask tiles, the
three 96x64 W matrices, and the 8 transposed boundary rows per batch).
It returns a [48, 192] fp32 partial (rbase | y4 | y8); the host sums the
8 partials, applies the +-y corrections, and tiles rows to (B, S, E).
This nearly halves per-core HBM traffic vs loading full Wv per core
(the previous layout) and eliminates the long gather/broadcast tail.

Device pipeline per core (scalar issues all DMAs — its post-prologue
path is ~0.6 us faster than sync's, and sequential DMAs on one HWDGE
queue drain FIFO, so the first half's semaphore fires early):
  scalar : aux DMA, xc half DMAs (2), out DMA
  PE     : y4,y8 correction mms; 32 x (LDW + 12-col MM) base segment
           sums accumulated per batch in its own PSUM bank; one fused
           [96,48]@[96,64] rbase mm.  lhsT reads 128 cols (96 data + 32
           spill into the next tile) so the compiler's Fast-Weight-Load
           kicks in; spill lands in psum partitions 96:127, never read.
  DVE    : psum->sbuf bf16 casts of Zb.T, fp32 copies of the outputs.
"""

from contextlib import ExitStack

import ml_dtypes
import numpy as np

import concourse.bass as bass
from concourse import bacc, mybir
from concourse.bass_utils import run_bass_kernel_spmd

B, S, E, H, D = 4, 1024, 768, 12, 64
EW = 96              # e-slice width per core
NT = 32              # s-tiles of 128 rows (4 batches x 8)
XC = NT * EW         # 3072 xc columns
FP = mybir.dt.float32
BF = mybir.dt.bfloat16

LOB = [(1024 * n) // 12 for n in range(13)]          # base segment bounds
M4 = [LOB[n] for n in (1, 4, 7, 10)]                 # theta=4 boundary rows
M8 = [LOB[n] for n in (2, 5, 8, 11)]                 # theta=8 boundary rows

_CACHE = {}


def _build_nc():
    # Bass.__init__ unconditionally emits 4 const-tile memsets (gpsimd) and a
    # full all-engine barrier before user code; this kernel uses neither
    # (no const-bias activations, all cross-engine deps via explicit sems),
    # so suppress them during construction to shave NEFF startup time.
    _memset = bass.BassGpSimd.memset
    _barrier = bass.Bass.all_engine_barrier
    bass.BassGpSimd.memset = lambda self, ap, c: None
    bass.Bass.all_engine_barrier = lambda self, **kw: None
    try:
        nc = bacc.Bacc("TRN2", target_bir_lowering=False, debug=False,
                       enable_partition_id=False)
    finally:
        bass.BassGpSimd.memset = _memset
        bass.Bass.all_engine_barrier = _barrier

    xc_d = nc.dram_tensor("xc", [128, XC], BF, kind="ExternalInput").ap()
    aux_d = nc.dram_tensor("aux", [128, 320], BF, kind="ExternalInput").ap()
    out_d = nc.dram_tensor("out", [48, 192], FP, kind="ExternalOutput").ap()

    xc_sb = nc.alloc_sbuf_tensor("xc_sb", [128, XC], BF).ap()
    aux_sb = nc.alloc_sbuf_tensor("aux_sb", [128, 320], BF).ap()
    zbt_sb = nc.alloc_sbuf_tensor("zbt_sb", [96, 48], BF).ap()
    out_sb = nc.alloc_sbuf_tensor("out_sb", [48, 192], FP).ap()

    # aux column map
    UM = slice(0, 96)            # U mask tiles: col st*12+n
    WSUM = slice(96, 160)
    WP4 = slice(160, 224)
    WP8 = slice(224, 288)
    XR4 = slice(288, 304)        # col b*4+i, boundary row M4[i] of batch b
    XR8 = slice(304, 320)

    with ExitStack() as ctx:
        # one bank per batch: psum accumulation groups are tracked per 2KB
        # bank region, so concurrent per-batch chains must not share a bank
        pz = [ctx.enter_context(nc.psum_tensor(f"pz{b}", [128, 512], FP))
              for b in range(4)]
        pr = ctx.enter_context(nc.psum_tensor("pr", [128, 512], FP))
        daux = ctx.enter_context(nc.semaphore("daux"))
        dx = [ctx.enter_context(nc.semaphore(f"dx{i}")) for i in range(3)]
        dy = ctx.enter_context(nc.semaphore("dy"))
        dr = ctx.enter_context(nc.semaphore("dr"))
        pe_sem = ctx.enter_context(nc.semaphore("pe_sem"))
        dve_sem = ctx.enter_context(nc.semaphore("dve_sem"))
        dcopy = ctx.enter_context(nc.semaphore("dcopy"))
        ycopy = ctx.enter_context(nc.semaphore("ycopy"))
        block = ctx.enter_context(nc.Block(no_gpsimd_drain=True))

        # xc chunks (in tile units): b0+b1 | b2 | b3 — finer chunks at the
        # tail so the last batch's PE work starts as soon as ITS bytes land
        CH = [(0, 16), (16, 24), (24, 32)]

        @block.scalar
        def _(scalar: bass.BassEngine):
            scalar.dma_start(aux_sb, aux_d).then_inc(daux, 16)
            for i, (lo, hi) in enumerate(CH):
                scalar.dma_start(xc_sb[:, lo * EW:hi * EW],
                                 xc_d[:, lo * EW:hi * EW]).then_inc(dx[i], 16)
            scalar.wait_ge(dcopy, 1)
            scalar.dma_start(out_d[0:48, 0:64], out_sb[0:48, 0:64]
                             ).then_inc(dr, 16)
            scalar.wait_ge(dr, 16)

        @block.sync
        def _(sync: bass.BassEngine):
            # y-correction half of the output is ready early — ship it
            # while the input stream is still running
            sync.wait_ge(ycopy, 1)
            sync.dma_start(out_d[0:16, 64:192], out_sb[0:16, 64:192]
                           ).then_inc(dy, 16)
            sync.wait_ge(dy, 16)

        @block.tensor
        def _(tensor: bass.BassEngine):
            tensor.wait_ge(daux, 16)
            # corrections first: depend only on aux
            nc.tensor.matmul(pr[0:16, 256:320], aux_sb[0:96, XR4],
                             aux_sb[0:96, WP4], start=True, stop=True)
            nc.tensor.matmul(pr[0:16, 320:384], aux_sb[0:96, XR8],
                             aux_sb[0:96, WP8], start=True, stop=True
                             ).then_inc(pe_sem)                    # pe=1
            # base segment sums Zb.T, accumulated per batch
            for b in range(4):
                if b == 0:
                    tensor.wait_ge(dx[0], 16)
                elif b == 2:
                    tensor.wait_ge(dx[1], 16)
                elif b == 3:
                    tensor.wait_ge(dx[2], 16)
                for st in range(8):
                    t = b * 8 + st
                    # 128-wide lhsT (32-col spill into the next tile) turns
                    # on FWL; the spill only pollutes psum partitions
                    # 96:127, which are never read.  The last tile of each
                    # DMA chunk must not spill across the chunk boundary.
                    w = 96 if t in (15, 23, 31) else 128
                    mm = nc.tensor.matmul(
                        pz[b][0:w, 0:12],
                        xc_sb[:, t * EW:t * EW + w],
                        aux_sb[:, st * 12:(st + 1) * 12],
                        start=(st == 0), stop=(st == 7))
                    if st == 7:
                        mm.then_inc(pe_sem)                        # pe=2+b
            # rbase for all 4 batches in one mm: out rows (b*12+n)
            tensor.wait_ge(dve_sem, 4)
            nc.tensor.matmul(pr[0:48, 0:64], zbt_sb[:, 0:48],
                             aux_sb[0:96, WSUM], start=True, stop=True
                             ).then_inc(pe_sem)                    # pe=6

        @block.vector
        def _(vector: bass.BassEngine):
            vector.wait_ge(pe_sem, 1)
            nc.vector.tensor_copy(out_sb[0:16, 64:192], pr[0:16, 256:384]
                                  ).then_inc(ycopy)
            for b in range(4):
                vector.wait_ge(pe_sem, 2 + b)
                nc.vector.tensor_copy(zbt_sb[:, b * 12:(b + 1) * 12],
                                      pz[b][0:96, 0:12]
                                      ).then_inc(dve_sem)
            vector.wait_ge(pe_sem, 6)
            nc.vector.tensor_copy(out_sb[0:48, 0:64], pr[0:48, 0:64]
                                  ).then_inc(dcopy)

    nc.compile()
    return nc


def _get_nc():
    if "nc" not in _CACHE:
        _CACHE["nc"] = _build_nc()
    return _CACHE["nc"]


def _umask_tiles() -> np.ndarray:
    """um[p, st*12+n] = 1 iff base segment n contains row st*128+p."""
    um = np.zeros((128, 96), np.float32)
    for st in range(8):
        for n in range(12):
            for p in range(128):
                s = st * 128 + p
                if LOB[n] <= s < LOB[n + 1]:
                    um[p, st * 12 + n] = 1.0
    return um


def _in_maps(inputs):
    v = np.asarray(inputs["value"], dtype=np.float32)
    WT = np.asarray(inputs["Wv"], np.float32).T          # [E, E]
    Wg = WT.reshape(E, 12, 64)
    wsum = Wg.sum(1)
    wp4 = Wg[:, :4, :].sum(1)
    wp8 = Wg[:, :8, :].sum(1)
    um = _umask_tiles()

    maps = []
    for k in range(8):
        sl = slice(k * EW, (k + 1) * EW)
        # xc[p, (b*8+st)*96 + e] = value[b, st*128+p, 96k+e]
        xc = (v[:, :, sl].reshape(4, 8, 128, EW)
              .transpose(2, 0, 1, 3).reshape(128, XC))
        aux = np.zeros((128, 320), np.float32)
        aux[:, 0:96] = um
        aux[0:96, 96:160] = wsum[sl]
        aux[0:96, 160:224] = wp4[sl]
        aux[0:96, 224:288] = wp8[sl]
        # xr: col b*4+i = boundary row M[i] of batch b, e on partitions
        aux[0:96, 288:304] = v[:, M4, sl].reshape(16, EW).T
        aux[0:96, 304:320] = v[:, M8, sl].reshape(16, EW).T
        maps.append({
            "xc": np.ascontiguousarray(xc).astype(ml_dtypes.bfloat16),
            "aux": aux.astype(ml_dtypes.bfloat16),
        })
    return maps


def _assemble(results):
    # sum the 8 e-slice partials, then apply boundary corrections
    acc = np.zeros((48, 192), np.float64)
    for k in range(8):
        acc += results[k]["out"].astype(np.float64)
    rbase = acc[0:48, 0:64].reshape(4, 12, 64)           # [b, n, d]
    y4 = acc[0:16, 64:128].reshape(4, 4, 64)             # [b, i, d]
    y8 = acc[0:16, 128:192].reshape(4, 4, 64)

    r = rbase.copy()
    for i, n in enumerate((1, 4, 7, 10)):
        r[:, n] -= y4[:, i]
        r[:, n - 1] += y4[:, i]
    for i, n in enumerate((2, 5, 8, 11)):
        r[:, n] -= y8[:, i]
        r[:, n - 1] += y8[:, i]

    row = r.reshape(B, E).astype(np.float32)             # [b, 768]
    out = np.empty((B, S, E), np.float32)
    out[:] = row[:, None, :]
    return out


def run(inputs, trace=False, **kw):
    """Run on hardware; returns (full_output, BassKernelResults)."""
    nc = _get_nc()
    res = run_bass_kernel_spmd(nc, _in_maps(inputs), core_ids=list(range(8)),
                               trace=trace, **kw)
    return _assemble(res.results), res


def kernel(**inputs) -> np.ndarray:
    out, _ = run(inputs)
    return out


# revision 15
# speedup vs baseline: 1.2203x; 1.0324x over previous
"""nn_Attention_19121194402320 on 8 TRN2 NeuronCores (raw Bass, bf16).

The reference module is

    k = (key @ Wk.T).reshape(B, H, S, D)       # RAW reshape
    q, v analogously
    attn = softmax(q @ k.T, axis=-1)
    out  = einsum('bnqk,bnvd->bnqd', attn, v)  # NOTE the 'k' vs 'v' labels
    out.transpose(0,2,1,3).reshape(B, S, E)

The second einsum's contraction labels differ ('k' in the first operand,
'v' in the second), so einsum sums each independently:

    out[b,n,q,d] = (sum_k attn[b,n,q,k]) * (sum_v v[b,n,v,d])
                 = sum_v v[b,n,v,d]          (softmax rows sum to 1)

i.e. every output row (for any q) equals the per-head column-sum of the
raw-reshaped V projection; query/key/Wq/Wk do not affect the output.

Math: with Y = value[b] @ Wv.T ([1024, 768]), raw-reshape head n covers
flat chunks g in [1024n, 1024(n+1)); chunk g = 12s + c is Y[s, 64c:64c+64].
So r_b[64n+d] = sum_c sum_{s in S(n,c)} Y[s, 64c+d] where S(n,c) =
[ceil((1024n-c)/12), ceil((1024(n+1)-c)/12)).  The boundary of S(n,.) as a
function of c moves by AT MOST ONE ROW: lo(n,c) = m_n + [c < theta_n] with
m_n = floor(1024n/12), theta_n = 4 if n%3==1, 8 if n%3==2, else no shift.
Hence with base segments [m_n, m_{n+1}) (indicator U [1024, 12]):

    Zb[n,:]  = sum_{s in base seg n} X[s,:]
    rbase    = Zb @ Wsum,   Wsum[e,d]   = sum_{c<12} Wv.T[e, 64c+d]
    y_n      = X[m_n] @ Wpre_{theta_n}, Wpre_t[e,d] = sum_{c<t} Wv.T[e, 64c+d]
    r[n]     = rbase[n] - y_n*[n has bnd] + y_{n+1}*[n+1 has bnd]

(verified to 3e-7 vs the fp32 jax reference).

Sharding: by the contraction dim e — core k owns e-slice [96k, 96k+96).
Each core loads: its column slice of value for ALL 4 batches (host-packed
[128, 32*96] bf16, 786 KB), plus an 80 KB aux tensor (U mask tiles, the
three 96x64 W matrices, and the 8 transposed boundary rows per batch).
It returns a [48, 192] fp32 partial (rbase | y4 | y8); the host sums the
8 partials, applies the +-y corrections, and tiles rows to (B, S, E).
This nearly halves per-core HBM traffic vs loading full Wv per core
(the previous layout) and eliminates the long gather/broadcast tail.

Device pipeline per core (scalar issues all DMAs — its post-prologue
path is ~0.6 us faster than sync's, and sequential DMAs on one HWDGE
queue drain FIFO, so the first half's semaphore fires early):
  scalar : aux DMA, xc half DMAs (2), out DMA
  PE     : y4,y8 correction mms; 32 x (LDW + 12-col MM) base segment
           sums accumulated per batch in its own PSUM bank; one fused
           [96,48]@[96,64] rbase mm.  lhsT reads 128 cols (96 data + 32
           spill into the next tile) so the compiler's Fast-Weight-Load
           kicks in; spill lands in psum partitions 96:127, never read.
  DVE    : psum->sbuf bf16 casts of Zb.T, fp32 copies of the outputs.
"""

from contextlib import ExitStack

import ml_dtypes
import numpy as np

import concourse.bass as bass
from concourse import bacc, mybir
from concourse.bass_utils import run_bass_kernel_spmd

B, S, E, H, D = 4, 1024, 768, 12, 64
EW = 96              # e-slice width per core
NT = 32              # s-tiles of 128 rows (4 batches x 8)
XC = NT * EW         # 3072 xc columns
FP = mybir.dt.float32
BF = mybir.dt.bfloat16

LOB = [(1024 * n) // 12 for n in range(13)]          # base segment bounds
M4 = [LOB[n] for n in (1, 4, 7, 10)]                 # theta=4 boundary rows
M8 = [LOB[n] for n in (2, 5, 8, 11)]                 # theta=8 boundary rows

_CACHE = {}


def _build_nc():
    # Bass.__init__ unconditionally emits 4 const-tile memsets (gpsimd) and a
    # full all-engine barrier before user code; this kernel uses neither
    # (no const-bias activations, all cross-engine deps via explicit sems),
    # so suppress them during construction to shave NEFF startup time.
    _memset = bass.BassGpSimd.memset
    _barrier = bass.Bass.all_engine_barrier
    bass.BassGpSimd.memset = lambda self, ap, c: None
    bass.Bass.all_engine_barrier = lambda self, **kw: None
    try:
        nc = bacc.Bacc("TRN2", target_bir_lowering=False, debug=False,
                       enable_partition_id=False)
    finally:
        bass.BassGpSimd.memset = _memset
        bass.Bass.all_engine_barrier = _barrier

    xc_d = nc.dram_tensor("xc", [128, XC], BF, kind="ExternalInput").ap()
    aux_d = nc.dram_tensor("aux", [128, 320], BF, kind="ExternalInput").ap()
    out_d = nc.dram_tensor("out", [48, 192], FP, kind="ExternalOutput").ap()

    xc_sb = nc.alloc_sbuf_tensor("xc_sb", [128, XC], BF).ap()
    aux_sb = nc.alloc_sbuf_tensor("aux_sb", [128, 320], BF).ap()
    zbt_sb = nc.alloc_sbuf_tensor("zbt_sb", [96, 48], BF).ap()
    out_sb = nc.alloc_sbuf_tensor("out_sb", [48, 192], FP).ap()

    # aux column map
    UM = slice(0, 96)            # U mask tiles: col st*12+n
    WSUM = slice(96, 160)
    WP4 = slice(160, 224)
    WP8 = slice(224, 288)
    XR4 = slice(288, 304)        # col b*4+i, boundary row M4[i] of batch b
    XR8 = slice(304, 320)

    with ExitStack() as ctx:
        # one bank per batch: psum accumulation groups are tracked per 2KB
        # bank region, so concurrent per-batch chains must not share a bank
        pz = [ctx.enter_context(nc.psum_tensor(f"pz{b}", [128, 512], FP))
              for b in range(4)]
        pr = ctx.enter_context(nc.psum_tensor("pr", [128, 512], FP))
        daux = ctx.enter_context(nc.semaphore("daux"))
        dx = [ctx.enter_context(nc.semaphore(f"dx{i}")) for i in range(3)]
        dy = ctx.enter_context(nc.semaphore("dy"))
        dr = ctx.enter_context(nc.semaphore("dr"))
        pe_sem = ctx.enter_context(nc.semaphore("pe_sem"))
        dve_sem = ctx.enter_context(nc.semaphore("dve_sem"))
        dcopy = ctx.enter_context(nc.semaphore("dcopy"))
        ycopy = ctx.enter_context(nc.semaphore("ycopy"))
        block = ctx.enter_context(nc.Block(no_gpsimd_drain=True))

        # xc chunks (in tile units): b0+b1 | b2 | b3 — finer chunks at the
        # tail so the last batch's PE work starts as soon as ITS bytes land
        CH = [(0, 16), (16, 24), (24, 32)]

        @block.scalar
        def _(scalar: bass.BassEngine):
            # aux + first xc chunk on scalar (fastest out of the preamble);
            # the other two chunks issue in parallel from sync so the SDMA
            # ring fills ~1 us earlier
            scalar.dma_start(aux_sb, aux_d).then_inc(daux, 16)
            lo, hi = CH[0]
            scalar.dma_start(xc_sb[:, lo * EW:hi * EW],
                             xc_d[:, lo * EW:hi * EW]).then_inc(dx[0], 16)
            scalar.wait_ge(dcopy, 1)
            scalar.dma_start(out_d[0:48, 0:64], out_sb[0:48, 0:64]
                             ).then_inc(dr, 16)
            scalar.wait_ge(dr, 16)

        @block.sync
        def _(sync: bass.BassEngine):
            for i in (1, 2):
                lo, hi = CH[i]
                sync.dma_start(xc_sb[:, lo * EW:hi * EW],
                               xc_d[:, lo * EW:hi * EW]).then_inc(dx[i], 16)
            # y-correction half of the output is ready early — ship it
            # while the input stream is still running
            sync.wait_ge(ycopy, 1)
            sync.dma_start(out_d[0:16, 64:192], out_sb[0:16, 64:192]
                           ).then_inc(dy, 16)
            sync.wait_ge(dy, 16)

        @block.tensor
        def _(tensor: bass.BassEngine):
            tensor.wait_ge(daux, 16)
            # corrections first: depend only on aux
            nc.tensor.matmul(pr[0:16, 256:320], aux_sb[0:96, XR4],
                             aux_sb[0:96, WP4], start=True, stop=True)
            nc.tensor.matmul(pr[0:16, 320:384], aux_sb[0:96, XR8],
                             aux_sb[0:96, WP8], start=True, stop=True
                             ).then_inc(pe_sem)                    # pe=1
            # base segment sums Zb.T, accumulated per batch
            for b in range(4):
                if b == 0:
                    tensor.wait_ge(dx[0], 16)
                elif b == 2:
                    tensor.wait_ge(dx[1], 16)
                elif b == 3:
                    tensor.wait_ge(dx[2], 16)
                for st in range(8):
                    t = b * 8 + st
                    # 128-wide lhsT (32-col spill into the next tile) turns
                    # on FWL; the spill only pollutes psum partitions
                    # 96:127, which are never read.  The last tile of each
                    # DMA chunk must not spill across the chunk boundary.
                    w = 96 if t in (15, 23, 31) else 128
                    mm = nc.tensor.matmul(
                        pz[b][0:w, 0:12],
                        xc_sb[:, t * EW:t * EW + w],
                        aux_sb[:, st * 12:(st + 1) * 12],
                        start=(st == 0), stop=(st == 7))
                    if st == 7:
                        mm.then_inc(pe_sem)                        # pe=2+b
            # rbase for all 4 batches in one mm: out rows (b*12+n)
            tensor.wait_ge(dve_sem, 4)
            nc.tensor.matmul(pr[0:48, 0:64], zbt_sb[:, 0:48],
                             aux_sb[0:96, WSUM], start=True, stop=True
                             ).then_inc(pe_sem)                    # pe=6

        @block.vector
        def _(vector: bass.BassEngine):
            vector.wait_ge(pe_sem, 1)
            nc.vector.tensor_copy(out_sb[0:16, 64:192], pr[0:16, 256:384]
                                  ).then_inc(ycopy)
            for b in range(4):
                vector.wait_ge(pe_sem, 2 + b)
                nc.vector.tensor_copy(zbt_sb[:, b * 12:(b + 1) * 12],
                                      pz[b][0:96, 0:12]
                                      ).then_inc(dve_sem)
            vector.wait_ge(pe_sem, 6)
            nc.vector.tensor_copy(out_sb[0:48, 0:64], pr[0:48, 0:64]
                                  ).then_inc(dcopy)

    nc.compile()
    return nc


def _get_nc():
    if "nc" not in _CACHE:
        _CACHE["nc"] = _build_nc()
    return _CACHE["nc"]


def _umask_tiles() -> np.ndarray:
    """um[p, st*12+n] = 1 iff base segment n contains row st*128+p."""
    um = np.zeros((128, 96), np.float32)
    for st in range(8):
        for n in range(12):
            for p in range(128):
                s = st * 128 + p
                if LOB[n] <= s < LOB[n + 1]:
                    um[p, st * 12 + n] = 1.0
    return um


def _in_maps(inputs):
    v = np.asarray(inputs["value"], dtype=np.float32)
    WT = np.asarray(inputs["Wv"], np.float32).T          # [E, E]
    Wg = WT.reshape(E, 12, 64)
    wsum = Wg.sum(1)
    wp4 = Wg[:, :4, :].sum(1)
    wp8 = Wg[:, :8, :].sum(1)
    um = _umask_tiles()

    maps = []
    for k in range(8):
        sl = slice(k * EW, (k + 1) * EW)
        # xc[p, (b*8+st)*96 + e] = value[b, st*128+p, 96k+e]
        xc = (v[:, :, sl].reshape(4, 8, 128, EW)
              .transpose(2, 0, 1, 3).reshape(128, XC))
        aux = np.zeros((128, 320), np.float32)
        aux[:, 0:96] = um
        aux[0:96, 96:160] = wsum[sl]
        aux[0:96, 160:224] = wp4[sl]
        aux[0:96, 224:288] = wp8[sl]
        # xr: col b*4+i = boundary row M[i] of batch b, e on partitions
        aux[0:96, 288:304] = v[:, M4, sl].reshape(16, EW).T
        aux[0:96, 304:320] = v[:, M8, sl].reshape(16, EW).T
        maps.append({
            "xc": np.ascontiguousarray(xc).astype(ml_dtypes.bfloat16),
            "aux": aux.astype(ml_dtypes.bfloat16),
        })
    return maps


def _assemble(results):
    # sum the 8 e-slice partials, then apply boundary corrections
    acc = np.zeros((48, 192), np.float64)
    for k in range(8):
        acc += results[k]["out"].astype(np.float64)
    rbase = acc[0:48, 0:64].reshape(4, 12, 64)           # [b, n, d]
    y4 = acc[0:16, 64:128].reshape(4, 4, 64)             # [b, i, d]
    y8 = acc[0:16, 128:192].reshape(4, 4, 64)

    r = rbase.copy()
    for i, n in enumerate((1, 4, 7, 10)):
        r[:, n] -= y4[:, i]
        r[:, n - 1] += y4[:, i]
    for i, n in enumerate((2, 5, 8, 11)):
        r[:, n] -= y8[:, i]
        r[:, n - 1] += y8[:, i]

    row = r.reshape(B, E).astype(np.float32)             # [b, 768]
    out = np.empty((B, S, E), np.float32)
    out[:] = row[:, None, :]
    return out


def run(inputs, trace=False, **kw):
    """Run on hardware; returns (full_output, BassKernelResults)."""
    nc = _get_nc()
    res = run_bass_kernel_spmd(nc, _in_maps(inputs), core_ids=list(range(8)),
                               trace=trace, **kw)
    return _assemble(res.results), res


def kernel(**inputs) -> np.ndarray:
    out, _ = run(inputs)
    return out
